# revision 12
# baseline (speedup 1.0000x reference)
"""AdaptiveVectorQuantizer Trainium2 kernel (8 NeuronCores, data-parallel).

Layout insight: BCHW input means each image b is natively [C=64, H*W=4096] =
channels-on-partitions, tokens-on-free — exactly the xT layout the PE wants,
and the output [L, B, C, H, W] is also channel-major, so gather results
[64, T] DMA out with 2KB-contiguous runs. No transposes of bulk data.

Per core: 4 images. Per image (4096 tokens, 32 tiles of 128):
 1. PE: score psum [128 tok, 256 codes] = e_sq - 2 x.e  (K=65 matmul: x rows
    + a ones row; rhs = [-2*cbT ; e_sq]).  argmin(score) == argmin dist.
 2. ACT copies psum -> packed SBUF keybuf [128, 32, 256].
 3. DVE/GP: per dyadic block b (sizes 2,2,4,..,128 covering codes 0..255):
    block-min (free-axis reduce), eq-mask, mask*iota_neg, reduce -> block
    argmin (first-occurrence ties).  Running chain over blocks yields
    per-level argmin idx_l (levels l: prefix k=2^(l+1)).
 4. PE transpose bridges idx (+idx^2) to rows; K=32 selector matmul computes
    g2 = idx^2 - 2*code*idx per combined-onehot row; threshold
    Relu(1-code^2 - g2) / is_lt(0.5-code^2) gives exact {0,1} one-hots.
 5. Block-diagonal gather matmuls (zero rows pad to 32-aligned slices)
    produce channel-major outputs for level pairs; psum->sbuf; DMA out.

Combined-onehot tiles (rows must be 32-aligned for engine slicing):
  T1: rows 0-1 l0 | 2-5 l1 | 6-13 l2 | 14-29 l3 | 30-31 pad | 32-63 l4
      | 64-127 l5
  T2: l6 codes 0-127.  T3: l7 codes 0-127.  T4: l7 codes 128-255.
"""

import sys, os
import numpy as np

sys.path.insert(0, "/opt/trn_rl_repo")

B, C, H, W = 32, 64, 64, 64
P, D = 256, 64
NCORES = 8
IMGS = B // NCORES            # 4 images per core
HWTOK = H * W                 # 4096 tokens per image
NTILE = HWTOK // 128          # 32
NSPAN = HWTOK // 512          # 8
NLVL = 8

# T1 row layout: list of (level, code) per row; pad rows get level -1
# rows 0-31: l4; rows 32-95: l5; rows 96-125: l0..l3; 126-127 pad
_T1_ROWS = [(4, j) for j in range(32)]
_T1_ROWS += [(5, j) for j in range(64)]
for lvl, k in ((0, 2), (1, 4), (2, 8), (3, 16)):
    _T1_ROWS += [(lvl, j) for j in range(k)]
_T1_ROWS += [(-1, 0), (-1, 0)]                     # rows 126-127 pad
assert len(_T1_ROWS) == 128


def _tile_maps():
    t1l = np.array([r[0] for r in _T1_ROWS])
    t1c = np.array([r[1] for r in _T1_ROWS])
    return [
        (t1l, t1c),
        (np.full(128, 6), np.arange(128)),            # T2: level 6
        (np.full(128, 7), np.arange(128)),            # T3: level 7 lo
        (np.full(128, 7), np.arange(128) + 128),      # T4: level 7 hi
    ]


def _host_consts(codebook):
    cb = np.ascontiguousarray(codebook, dtype=np.float32)         # [256, 64]
    esq = np.sum(cb * cb, axis=1, dtype=np.float32)               # [256]
    cbt2 = np.ascontiguousarray(-2.0 * cb.T, dtype=np.float32)    # [64, 256]
    esqrow = np.ascontiguousarray(esq[None, :])                   # [1, 256]
    onescol = np.ones((1, 128), np.float32)

    iota_neg = np.broadcast_to(
        (np.arange(256, dtype=np.float32) - 512.0)[None, :], (128, 256)
    ).copy()

    selqs, biases, lts = [], [], []
    for rowlevel, rowcode in _tile_maps():
        # selq rows interleave (idx_l, idx_l^2) coeffs at rows 2l, 2l+1;
        # rows 16-31 unused (transpose pad) -> zero.
        sq = np.zeros((32, 128), np.float32)
        for p_ in range(128):
            l = rowlevel[p_]
            if l >= 0:
                sq[2 * l, p_] = -2.0 * rowcode[p_]   # coeff of idx_l
                sq[2 * l + 1, p_] = 1.0              # coeff of idx_l^2
        selqs.append(sq)
        code = rowcode.astype(np.float32).copy()
        code[rowlevel < 0] = 2.0   # pad sentinel: g2=0 -> 1-4<0 -> oh=0
        biases.append((1.0 - code * code).astype(np.float32))     # ACT Relu
        lts.append((0.5 - code * code).astype(np.float32))        # DVE is_lt

    # gather lhsT weights (zero rows where the onehot row isn't contracted)
    g01 = np.zeros((128, 128), np.float32)   # rows 96-127 used (base match)
    g01[96:98, 0:64] = cb[0:2]        # l0 -> out cols 0:64
    g01[98:102, 64:128] = cb[0:4]     # l1 -> out cols 64:128
    g23 = np.zeros((128, 128), np.float32)
    g23[102:110, 0:64] = cb[0:8]      # l2
    g23[110:126, 64:128] = cb[0:16]   # l3
    g45 = np.zeros((96, 128), np.float32)
    g45[0:32, 0:64] = cb[0:32]        # l4 (T1 rows 0-31)
    g45[32:96, 64:128] = cb[0:64]     # l5 (T1 rows 32-95)
    cbc0 = np.ascontiguousarray(cb[0:128])    # [128, 64]
    cbc1 = np.ascontiguousarray(cb[128:256])  # [128, 64]
    ident = np.eye(128, dtype=np.float32)
    return dict(
        cbt2=cbt2, esqrow=esqrow, onescol=onescol, iota_neg=iota_neg,
        selq1=selqs[0], selq2=selqs[1], selq3=selqs[2], selq4=selqs[3],
        bias1=biases[0].reshape(128, 1), bias2=biases[1].reshape(128, 1),
        lt3=lts[2].reshape(128, 1), lt4=lts[3].reshape(128, 1),
        g01=g01, g23=g23, g45=g45, cbc0=cbc0, cbc1=cbc1,
        ident=ident,
    )


_CACHE = {}

# block b covers codes [blo, bhi)
_BLOCKS = [(0, 2)] + [(1 << b, 1 << (b + 1)) for b in range(1, 8)]


def _build():
    if "nc" in _CACHE:
        return _CACHE["nc"]
    from concourse import bass, bacc, tile, mybir

    f32 = mybir.dt.float32
    Alu = mybir.AluOpType
    Act = mybir.ActivationFunctionType
    AxX = mybir.AxisListType.X

    nc = bacc.Bacc("TRN2", target_bir_lowering=False, debug=False,
                   num_devices=NCORES)
    x_d = nc.declare_dram_parameter("x", [IMGS, 64, HWTOK], f32, isOutput=False)
    consts_spec = [
        ("cbt2", [64, 256]), ("esqrow", [1, 256]), ("onescol", [1, 128]),
        ("iota_neg", [128, 256]),
        ("selq1", [32, 128]), ("selq2", [32, 128]),
        ("selq3", [32, 128]), ("selq4", [32, 128]),
        ("bias1", [128, 1]), ("bias2", [128, 1]),
        ("lt3", [128, 1]), ("lt4", [128, 1]),
        ("g01", [128, 128]), ("g23", [128, 128]), ("g45", [96, 128]),
        ("cbc0", [128, 64]), ("cbc1", [128, 64]),
        ("ident", [128, 128]),
    ]
    cd = {name: nc.declare_dram_parameter(name, shp, f32, isOutput=False)
          for name, shp in consts_spec}
    out_d = nc.declare_dram_parameter("out", [NLVL, IMGS, 64, HWTOK], f32,
                                      isOutput=True)

    with tile.TileContext(nc) as tc:
        with (
            tc.tile_pool(name="const", bufs=1) as cpool,
            tc.tile_pool(name="xp", bufs=2) as xpool,
            tc.tile_pool(name="kb", bufs=2) as kbpool,
            tc.tile_pool(name="scr", bufs=1) as scrpool,
            tc.tile_pool(name="small", bufs=2) as smpool,
            tc.tile_pool(name="ohp", bufs=2) as ohpool,
            tc.tile_pool(name="outp", bufs=3) as outpool,
            tc.tile_pool(name="kps", bufs=2, space="PSUM") as kpsum,
            tc.tile_pool(name="tps", bufs=1, space="PSUM") as tpsum,
            tc.tile_pool(name="gqs", bufs=2, space="PSUM") as gqpsum,
            tc.tile_pool(name="gos", bufs=3, space="PSUM") as gopsum,
        ):
            cs = {}
            for name, shp in consts_spec:
                t = cpool.tile(shp, f32, tag=name)
                nc.sync.dma_start(out=t[:], in_=cd[name][:])
                cs[name] = t

            for img in range(IMGS):
                xT = xpool.tile([64, HWTOK], f32)
                nc.sync.dma_start(out=xT[:], in_=x_d[img])

                keybuf = kbpool.tile([128, NTILE, 256], f32)
                for t in range(NTILE):
                    kp = kpsum.tile([128, 256], f32)
                    nc.tensor.matmul(
                        kp[:], xT[:, t * 128:(t + 1) * 128], cs["cbt2"][:],
                        start=True, stop=False,
                    )
                    nc.tensor.matmul(
                        kp[:], cs["onescol"][:], cs["esqrow"][:],
                        start=False, stop=True,
                    )
                    nc.scalar.copy(keybuf[:, t, :], kp[:])

                bm = smpool.tile([128, 8, NTILE], f32, tag="bm")
                bidx = smpool.tile([128, 8, NTILE], f32, tag="bidx")
                eqs = scrpool.tile([128, NTILE, 128], f32)
                for b, (blo, bhi) in enumerate(_BLOCKS):
                    h = bhi - blo
                    nc.vector.tensor_reduce(
                        out=bm[:, b, :], in_=keybuf[:, :, blo:bhi],
                        axis=AxX, op=Alu.min,
                    )
                    bmb = bm[:, b, :].unsqueeze(2).to_broadcast([128, NTILE, h])
                    nc.vector.tensor_tensor(
                        out=eqs[:, :, 0:h], in0=keybuf[:, :, blo:bhi],
                        in1=bmb, op=Alu.is_equal,
                    )
                    iot = cs["iota_neg"][:, blo:bhi].unsqueeze(1).to_broadcast(
                        [128, NTILE, h])
                    nc.vector.tensor_tensor(
                        out=eqs[:, :, 0:h], in0=eqs[:, :, 0:h], in1=iot,
                        op=Alu.mult,
                    )
                    nc.vector.tensor_reduce(
                        out=bidx[:, b, :], in_=eqs[:, :, 0:h],
                        axis=AxX, op=Alu.min,
                    )

                # running chain over blocks -> per-level idx (and idx^2),
                # written into 32-slot-per-tile layout for the transpose
                # (slots 2l = idx_l, 2l+1 = idx_l^2, 16..31 pad).
                idxq = smpool.tile([128, NTILE, 32], f32, tag="idxq")
                nc.gpsimd.memset(idxq[:], 0.0)
                runmin = bm[:, 0, :]
                runidx = bidx[:, 0, :]
                nc.vector.tensor_scalar(
                    out=idxq[:, :, 0], in0=bidx[:, 0, :],
                    scalar1=512.0, scalar2=None, op0=Alu.add,
                )
                for l in range(1, NLVL):
                    mask = smpool.tile([128, NTILE], mybir.dt.int32,
                                       tag="mask")
                    nm = smpool.tile([128, NTILE], f32, tag="nm")
                    ni = smpool.tile([128, NTILE], f32, tag="ni")
                    nc.vector.tensor_tensor(out=mask[:], in0=bm[:, l, :],
                                            in1=runmin, op=Alu.is_lt)
                    nc.vector.tensor_tensor(out=nm[:], in0=bm[:, l, :],
                                            in1=runmin, op=Alu.min)
                    nc.vector.tensor_copy(ni[:], runidx)
                    nc.vector.copy_predicated(ni[:], mask[:], bidx[:, l, :])
                    nc.vector.tensor_scalar(
                        out=idxq[:, :, 2 * l], in0=ni[:],
                        scalar1=512.0, scalar2=None, op0=Alu.add,
                    )
                    runmin, runidx = nm[:], ni[:]
                # squares into odd slots
                nc.vector.tensor_tensor(
                    out=idxq[:, :, 1:16:2], in0=idxq[:, :, 0:16:2],
                    in1=idxq[:, :, 0:16:2], op=Alu.mult,
                )

                for s in range(NSPAN):
                    tp = tpsum.tile([128, 128], f32)
                    nc.tensor.transpose(
                        tp[:], idxq[:, 4 * s:4 * s + 4, :], cs["ident"][:],
                    )
                    R = smpool.tile([32, 512], f32, tag="R")
                    for t in range(4):
                        nc.vector.tensor_copy(
                            R[0:32, t * 128:(t + 1) * 128],
                            tp[32 * t:32 * (t + 1), :],
                        )
                    ohs = {}
                    for X, selq in ((1, "selq1"), (2, "selq2"),
                                    (3, "selq3"), (4, "selq4")):
                        gq = gqpsum.tile([128, 512], f32)
                        nc.tensor.matmul(gq[:], cs[selq][:], R[:],
                                         start=True, stop=True)
                        oht = ohpool.tile([128, 512], f32, tag=f"oh{X}")
                        if X <= 2:
                            nc.scalar.activation(oht[:], gq[:], Act.Relu,
                                                 bias=cs[f"bias{X}"][:],
                                                 scale=-1.0)
                        else:
                            nc.vector.tensor_scalar(
                                out=oht[:], in0=gq[:], scalar1=cs[f"lt{X}"][:],
                                scalar2=None, op0=Alu.is_lt,
                            )
                        ohs[X] = oht

                    ps01 = gopsum.tile([128, 512], f32, tag="go")
                    ps23 = gopsum.tile([128, 512], f32, tag="go")
                    ps45 = gopsum.tile([128, 512], f32, tag="go")
                    ps67 = gopsum.tile([128, 512], f32, tag="go")
                    nc.tensor.matmul(ps01[:], cs["g01"][96:128, :],
                                     ohs[1][96:128, :],
                                     start=True, stop=True,
                                     tile_position=(96, 0))
                    nc.tensor.matmul(ps23[:], cs["g23"][96:128, :],
                                     ohs[1][96:128, :],
                                     start=True, stop=True,
                                     tile_position=(96, 0))
                    nc.tensor.matmul(ps45[:], cs["g45"][:], ohs[1][0:96, :],
                                     start=True, stop=True)
                    nc.tensor.matmul(ps67[0:64, :], cs["cbc0"][:],
                                     ohs[2][:], start=True, stop=True,
                                     tile_position=(0, 0))
                    nc.tensor.matmul(ps67[64:128, :], cs["cbc0"][:],
                                     ohs[3][:], start=True, stop=False,
                                     tile_position=(0, 64))
                    nc.tensor.matmul(ps67[64:128, :], cs["cbc1"][:],
                                     ohs[4][:], start=False, stop=True,
                                     tile_position=(0, 64))

                    for pi, (ps, eng) in enumerate(
                        ((ps01, "act"), (ps23, "act"),
                         (ps45, "dve"), (ps67, "dve"))
                    ):
                        cp = outpool.tile([128, 512], f32, tag=f"cp{pi}")
                        if eng == "act":
                            nc.scalar.copy(cp[:], ps[:])
                        else:
                            nc.vector.tensor_copy(cp[:], ps[:])
                        for half in range(2):
                            lvl = 2 * pi + half
                            nc.sync.dma_start(
                                out=out_d[lvl, img, :,
                                          s * 512:(s + 1) * 512],
                                in_=cp[64 * half:64 * (half + 1), :],
                            )
    nc.compile()
    _CACHE["nc"] = nc
    return nc


def kernel(input_data, codebook, previous_active_vectors=None,
           num_active_vectors=256, **_):
    from concourse.bass_utils import run_bass_kernel_spmd

    x = np.ascontiguousarray(np.asarray(input_data, dtype=np.float32))
    assert x.shape == (B, C, H, W)
    consts = _host_consts(np.asarray(codebook, dtype=np.float32))

    nc = _build()
    in_maps = []
    for core in range(NCORES):
        m = {"x": np.ascontiguousarray(
            x[core * IMGS:(core + 1) * IMGS].reshape(IMGS, 64, HWTOK))}
        m.update(consts)
        in_maps.append(m)
    res = run_bass_kernel_spmd(nc, in_maps, core_ids=list(range(NCORES)))
    outs = [res.results[i]["out"] for i in range(NCORES)]   # [8, 4, 64, 4096]
    full = np.concatenate(outs, axis=1)                     # [8, 32, 64, 4096]
    return full.reshape(NLVL, B, C, H, W)


# revision 13
# speedup vs baseline: 1.0315x; 1.0315x over previous
"""AdaptiveVectorQuantizer Trainium2 kernel (8 NeuronCores, data-parallel).

Layout insight: BCHW input means each image b is natively [C=64, H*W=4096] =
channels-on-partitions, tokens-on-free — exactly the xT layout the PE wants,
and the output [L, B, C, H, W] is also channel-major, so gather results
[64, T] DMA out with 2KB-contiguous runs. No transposes of bulk data.

Per core: 4 images. Per image (4096 tokens, 32 tiles of 128):
 1. PE: score psum [128 tok, 256 codes] = e_sq - 2 x.e  (K=65 matmul: x rows
    + a ones row; rhs = [-2*cbT ; e_sq]).  argmin(score) == argmin dist.
 2. ACT copies psum -> packed SBUF keybuf [128, 32, 256].
 3. DVE/GP: per dyadic block b (sizes 2,2,4,..,128 covering codes 0..255):
    block-min (free-axis reduce), eq-mask, mask*iota_neg, reduce -> block
    argmin (first-occurrence ties).  Running chain over blocks yields
    per-level argmin idx_l (levels l: prefix k=2^(l+1)).
 4. PE transpose bridges idx (+idx^2) to rows; K=32 selector matmul computes
    g2 = idx^2 - 2*code*idx per combined-onehot row; threshold
    Relu(1-code^2 - g2) / is_lt(0.5-code^2) gives exact {0,1} one-hots.
 5. Block-diagonal gather matmuls (zero rows pad to 32-aligned slices)
    produce channel-major outputs for level pairs; psum->sbuf; DMA out.

Combined-onehot tiles (rows must be 32-aligned for engine slicing):
  T1: rows 0-1 l0 | 2-5 l1 | 6-13 l2 | 14-29 l3 | 30-31 pad | 32-63 l4
      | 64-127 l5
  T2: l6 codes 0-127.  T3: l7 codes 0-127.  T4: l7 codes 128-255.
"""

import sys, os
import numpy as np

sys.path.insert(0, "/opt/trn_rl_repo")

B, C, H, W = 32, 64, 64, 64
P, D = 256, 64
NCORES = 8
IMGS = B // NCORES            # 4 images per core
HWTOK = H * W                 # 4096 tokens per image
NTILE = HWTOK // 128          # 32
NSPAN = HWTOK // 512          # 8
NLVL = 8

# T1 row layout: list of (level, code) per row; pad rows get level -1
# rows 0-31: l4; rows 32-95: l5; rows 96-125: l0..l3; 126-127 pad
_T1_ROWS = [(4, j) for j in range(32)]
_T1_ROWS += [(5, j) for j in range(64)]
for lvl, k in ((0, 2), (1, 4), (2, 8), (3, 16)):
    _T1_ROWS += [(lvl, j) for j in range(k)]
_T1_ROWS += [(-1, 0), (-1, 0)]                     # rows 126-127 pad
assert len(_T1_ROWS) == 128


def _tile_maps():
    t1l = np.array([r[0] for r in _T1_ROWS])
    t1c = np.array([r[1] for r in _T1_ROWS])
    return [
        (t1l, t1c),
        (np.full(128, 6), np.arange(128)),            # T2: level 6
        (np.full(128, 7), np.arange(128)),            # T3: level 7 lo
        (np.full(128, 7), np.arange(128) + 128),      # T4: level 7 hi
    ]


def _host_consts(codebook):
    cb = np.ascontiguousarray(codebook, dtype=np.float32)         # [256, 64]
    esq = np.sum(cb * cb, axis=1, dtype=np.float32)               # [256]
    cbt2e = np.concatenate([-2.0 * cb.T, esq[None, :]], axis=0)   # [65, 256]
    cbt2e = np.ascontiguousarray(cbt2e, dtype=np.float32)

    iota_neg = np.broadcast_to(
        (np.arange(256, dtype=np.float32) - 512.0)[None, :], (128, 256)
    ).copy()

    selqs, biases, lts = [], [], []
    for rowlevel, rowcode in _tile_maps():
        # selq rows interleave (idx_l, idx_l^2) coeffs at rows 2l, 2l+1;
        # rows 16-31 unused (transpose pad) -> zero.
        sq = np.zeros((32, 128), np.float32)
        for p_ in range(128):
            l = rowlevel[p_]
            if l >= 0:
                sq[2 * l, p_] = -2.0 * rowcode[p_]   # coeff of idx_l
                sq[2 * l + 1, p_] = 1.0              # coeff of idx_l^2
        selqs.append(sq)
        code = rowcode.astype(np.float32).copy()
        code[rowlevel < 0] = 2.0   # pad sentinel: g2=0 -> 1-4<0 -> oh=0
        biases.append((1.0 - code * code).astype(np.float32))     # ACT Relu
        lts.append((0.5 - code * code).astype(np.float32))        # DVE is_lt

    # gather lhsT weights (zero rows where the onehot row isn't contracted)
    g01 = np.zeros((128, 128), np.float32)   # rows 96-127 used (base match)
    g01[96:98, 0:64] = cb[0:2]        # l0 -> out cols 0:64
    g01[98:102, 64:128] = cb[0:4]     # l1 -> out cols 64:128
    g23 = np.zeros((128, 128), np.float32)
    g23[102:110, 0:64] = cb[0:8]      # l2
    g23[110:126, 64:128] = cb[0:16]   # l3
    g45 = np.zeros((96, 128), np.float32)
    g45[0:32, 0:64] = cb[0:32]        # l4 (T1 rows 0-31)
    g45[32:96, 64:128] = cb[0:64]     # l5 (T1 rows 32-95)
    cbc0 = np.ascontiguousarray(cb[0:128])    # [128, 64]
    cbc1 = np.ascontiguousarray(cb[128:256])  # [128, 64]
    ident = np.eye(128, dtype=np.float32)
    return dict(
        cbt2e=cbt2e, iota_neg=iota_neg,
        selq1=selqs[0], selq2=selqs[1], selq3=selqs[2], selq4=selqs[3],
        bias1=biases[0].reshape(128, 1), bias2=biases[1].reshape(128, 1),
        lt3=lts[2].reshape(128, 1), lt4=lts[3].reshape(128, 1),
        g01=g01, g23=g23, g45=g45, cbc0=cbc0, cbc1=cbc1,
        ident=ident,
    )


_CACHE = {}

# block b covers codes [blo, bhi)
_BLOCKS = [(0, 2)] + [(1 << b, 1 << (b + 1)) for b in range(1, 8)]


def _build():
    if "nc" in _CACHE:
        return _CACHE["nc"]
    from concourse import bass, bacc, tile, mybir

    f32 = mybir.dt.float32
    Alu = mybir.AluOpType
    Act = mybir.ActivationFunctionType
    AxX = mybir.AxisListType.X

    nc = bacc.Bacc("TRN2", target_bir_lowering=False, debug=False,
                   num_devices=NCORES)
    x_d = nc.declare_dram_parameter("x", [IMGS, 64, HWTOK], f32, isOutput=False)
    consts_spec = [
        ("cbt2e", [65, 256]), ("iota_neg", [128, 256]),
        ("selq1", [32, 128]), ("selq2", [32, 128]),
        ("selq3", [32, 128]), ("selq4", [32, 128]),
        ("bias1", [128, 1]), ("bias2", [128, 1]),
        ("lt3", [128, 1]), ("lt4", [128, 1]),
        ("g01", [128, 128]), ("g23", [128, 128]), ("g45", [96, 128]),
        ("cbc0", [128, 64]), ("cbc1", [128, 64]),
        ("ident", [128, 128]),
    ]
    cd = {name: nc.declare_dram_parameter(name, shp, f32, isOutput=False)
          for name, shp in consts_spec}
    out_d = nc.declare_dram_parameter("out", [NLVL, IMGS, 64, HWTOK], f32,
                                      isOutput=True)

    with tile.TileContext(nc) as tc:
        with (
            tc.tile_pool(name="const", bufs=1) as cpool,
            tc.tile_pool(name="xp", bufs=2) as xpool,
            tc.tile_pool(name="kb", bufs=2) as kbpool,
            tc.tile_pool(name="scr", bufs=1) as scrpool,
            tc.tile_pool(name="small", bufs=2) as smpool,
            tc.tile_pool(name="ohp", bufs=2) as ohpool,
            tc.tile_pool(name="outp", bufs=3) as outpool,
            tc.tile_pool(name="kps", bufs=2, space="PSUM") as kpsum,
            tc.tile_pool(name="tps", bufs=1, space="PSUM") as tpsum,
            tc.tile_pool(name="gqs", bufs=2, space="PSUM") as gqpsum,
            tc.tile_pool(name="gos", bufs=3, space="PSUM") as gopsum,
        ):
            cs = {}
            for name, shp in consts_spec:
                t = cpool.tile(shp, f32, tag=name)
                nc.sync.dma_start(out=t[:], in_=cd[name][:])
                cs[name] = t

            for img in range(IMGS):
                xT = xpool.tile([65, HWTOK], f32)
                nc.sync.dma_start(out=xT[0:64, :], in_=x_d[img])
                nc.gpsimd.memset(xT[64:65, :], 1.0)

                keybuf = kbpool.tile([128, NTILE, 256], f32)
                for tt in range(NTILE // 2):
                    kp = kpsum.tile([128, 512], f32)
                    for j in range(2):
                        t = 2 * tt + j
                        nc.tensor.matmul(
                            kp[:, j * 256:(j + 1) * 256],
                            xT[:, t * 128:(t + 1) * 128], cs["cbt2e"][:],
                            start=True, stop=True,
                        )
                    nc.scalar.copy(keybuf[:, 2 * tt:2 * tt + 2, :], kp[:])

                bm = smpool.tile([128, 8, NTILE], f32, tag="bm")
                bidx = smpool.tile([128, 8, NTILE], f32, tag="bidx")
                eqs = scrpool.tile([128, NTILE, 128], f32)
                for b, (blo, bhi) in enumerate(_BLOCKS):
                    h = bhi - blo
                    nc.vector.tensor_reduce(
                        out=bm[:, b, :], in_=keybuf[:, :, blo:bhi],
                        axis=AxX, op=Alu.min,
                    )
                    bmb = bm[:, b, :].unsqueeze(2).to_broadcast([128, NTILE, h])
                    nc.vector.tensor_tensor(
                        out=eqs[:, :, 0:h], in0=keybuf[:, :, blo:bhi],
                        in1=bmb, op=Alu.is_equal,
                    )
                    iot = cs["iota_neg"][:, blo:bhi].unsqueeze(1).to_broadcast(
                        [128, NTILE, h])
                    nc.gpsimd.tensor_tensor(
                        out=eqs[:, :, 0:h], in0=eqs[:, :, 0:h], in1=iot,
                        op=Alu.mult,
                    )
                    nc.vector.tensor_reduce(
                        out=bidx[:, b, :], in_=eqs[:, :, 0:h],
                        axis=AxX, op=Alu.min,
                    )

                # running chain over blocks -> per-level idx (and idx^2),
                # written into 32-slot-per-tile layout for the transpose
                # (slots 2l = idx_l, 2l+1 = idx_l^2, 16..31 pad).
                idxq = smpool.tile([128, NTILE, 32], f32, tag="idxq")
                nc.gpsimd.memset(idxq[:], 0.0)
                runmin = bm[:, 0, :]
                runidx = bidx[:, 0, :]
                nc.vector.tensor_scalar(
                    out=idxq[:, :, 0], in0=bidx[:, 0, :],
                    scalar1=512.0, scalar2=None, op0=Alu.add,
                )
                for l in range(1, NLVL):
                    mask = smpool.tile([128, NTILE], mybir.dt.int32,
                                       tag="mask")
                    nm = smpool.tile([128, NTILE], f32, tag="nm")
                    ni = smpool.tile([128, NTILE], f32, tag="ni")
                    nc.vector.tensor_tensor(out=mask[:], in0=bm[:, l, :],
                                            in1=runmin, op=Alu.is_lt)
                    nc.vector.tensor_tensor(out=nm[:], in0=bm[:, l, :],
                                            in1=runmin, op=Alu.min)
                    nc.vector.tensor_copy(ni[:], runidx)
                    nc.vector.copy_predicated(ni[:], mask[:], bidx[:, l, :])
                    nc.vector.tensor_scalar(
                        out=idxq[:, :, 2 * l], in0=ni[:],
                        scalar1=512.0, scalar2=None, op0=Alu.add,
                    )
                    runmin, runidx = nm[:], ni[:]
                # squares into odd slots
                nc.vector.tensor_tensor(
                    out=idxq[:, :, 1:16:2], in0=idxq[:, :, 0:16:2],
                    in1=idxq[:, :, 0:16:2], op=Alu.mult,
                )

                for s in range(NSPAN):
                    tp = tpsum.tile([128, 128], f32)
                    nc.tensor.transpose(
                        tp[:], idxq[:, 4 * s:4 * s + 4, :], cs["ident"][:],
                    )
                    R = smpool.tile([32, 512], f32, tag="R")
                    for t in range(4):
                        nc.vector.tensor_copy(
                            R[0:32, t * 128:(t + 1) * 128],
                            tp[32 * t:32 * (t + 1), :],
                        )
                    ohs = {}
                    for X, selq in ((1, "selq1"), (2, "selq2"),
                                    (3, "selq3"), (4, "selq4")):
                        gq = gqpsum.tile([128, 512], f32)
                        nc.tensor.matmul(gq[:], cs[selq][:], R[:],
                                         start=True, stop=True)
                        oht = ohpool.tile([128, 512], f32, tag=f"oh{X}")
                        if X <= 2:
                            nc.scalar.activation(oht[:], gq[:], Act.Relu,
                                                 bias=cs[f"bias{X}"][:],
                                                 scale=-1.0)
                        else:
                            nc.vector.tensor_scalar(
                                out=oht[:], in0=gq[:], scalar1=cs[f"lt{X}"][:],
                                scalar2=None, op0=Alu.is_lt,
                            )
                        ohs[X] = oht

                    ps01 = gopsum.tile([128, 512], f32, tag="go")
                    ps23 = gopsum.tile([128, 512], f32, tag="go")
                    ps45 = gopsum.tile([128, 512], f32, tag="go")
                    ps67 = gopsum.tile([128, 512], f32, tag="go")
                    nc.tensor.matmul(ps01[:], cs["g01"][96:128, :],
                                     ohs[1][96:128, :],
                                     start=True, stop=True,
                                     tile_position=(96, 0))
                    nc.tensor.matmul(ps23[:], cs["g23"][96:128, :],
                                     ohs[1][96:128, :],
                                     start=True, stop=True,
                                     tile_position=(96, 0))
                    nc.tensor.matmul(ps45[:], cs["g45"][:], ohs[1][0:96, :],
                                     start=True, stop=True)
                    nc.tensor.matmul(ps67[0:64, :], cs["cbc0"][:],
                                     ohs[2][:], start=True, stop=True,
                                     tile_position=(0, 0))
                    nc.tensor.matmul(ps67[64:128, :], cs["cbc0"][:],
                                     ohs[3][:], start=True, stop=False,
                                     tile_position=(0, 64))
                    nc.tensor.matmul(ps67[64:128, :], cs["cbc1"][:],
                                     ohs[4][:], start=False, stop=True,
                                     tile_position=(0, 64))

                    for pi, (ps, eng) in enumerate(
                        ((ps01, "act"), (ps23, "act"),
                         (ps45, "dve"), (ps67, "dve"))
                    ):
                        cp = outpool.tile([128, 512], f32, tag=f"cp{pi}")
                        if eng == "act":
                            nc.scalar.copy(cp[:], ps[:])
                        else:
                            nc.vector.tensor_copy(cp[:], ps[:])
                        for half in range(2):
                            lvl = 2 * pi + half
                            nc.sync.dma_start(
                                out=out_d[lvl, img, :,
                                          s * 512:(s + 1) * 512],
                                in_=cp[64 * half:64 * (half + 1), :],
                            )
    nc.compile()
    _CACHE["nc"] = nc
    return nc


def kernel(input_data, codebook, previous_active_vectors=None,
           num_active_vectors=256, **_):
    from concourse.bass_utils import run_bass_kernel_spmd

    x = np.ascontiguousarray(np.asarray(input_data, dtype=np.float32))
    assert x.shape == (B, C, H, W)
    consts = _host_consts(np.asarray(codebook, dtype=np.float32))

    nc = _build()
    in_maps = []
    for core in range(NCORES):
        m = {"x": np.ascontiguousarray(
            x[core * IMGS:(core + 1) * IMGS].reshape(IMGS, 64, HWTOK))}
        m.update(consts)
        in_maps.append(m)
    res = run_bass_kernel_spmd(nc, in_maps, core_ids=list(range(NCORES)))
    outs = [res.results[i]["out"] for i in range(NCORES)]   # [8, 4, 64, 4096]
    full = np.concatenate(outs, axis=1)                     # [8, 32, 64, 4096]
    return full.reshape(NLVL, B, C, H, W)


# revision 18
# speedup vs baseline: 1.7721x; 1.7181x over previous
"""AdaptiveVectorQuantizer Trainium2 kernel (8 NeuronCores, data-parallel).

Layout insight: BCHW input means each image b is natively [C=64, H*W=4096] =
channels-on-partitions, tokens-on-free — exactly the xT layout the PE wants,
and the output [L, B, C, H, W] is also channel-major, so gather results
[64, T] DMA out with 2KB-contiguous runs. No transposes of bulk data.

Per core: 4 images. Per image (4096 tokens, 32 tiles of 128):
 1. PE: score psum [128 tok, 256 codes] = e_sq - 2 x.e  (K=65 matmul: x rows
    + a ones row; rhs = [-2*cbT ; e_sq]).  argmin(score) == argmin dist.
 2. ACT copies psum -> packed SBUF keybuf [128, 32, 256].
 3. DVE/GP: per dyadic block b (sizes 2,2,4,..,128 covering codes 0..255):
    block-min (free-axis reduce), eq-mask, mask*iota_neg, reduce -> block
    argmin (first-occurrence ties).  Running chain over blocks yields
    per-level argmin idx_l (levels l: prefix k=2^(l+1)).
 4. PE transpose bridges idx (+idx^2) to rows; K=32 selector matmul computes
    g2 = idx^2 - 2*code*idx per combined-onehot row; threshold
    Relu(1-code^2 - g2) / is_lt(0.5-code^2) gives exact {0,1} one-hots.
 5. Block-diagonal gather matmuls (zero rows pad to 32-aligned slices)
    produce channel-major outputs for level pairs; psum->sbuf; DMA out.

Combined-onehot tiles (rows must be 32-aligned for engine slicing):
  T1: rows 0-1 l0 | 2-5 l1 | 6-13 l2 | 14-29 l3 | 30-31 pad | 32-63 l4
      | 64-127 l5
  T2: l6 codes 0-127.  T3: l7 codes 0-127.  T4: l7 codes 128-255.
"""

import sys, os
import numpy as np

sys.path.insert(0, "/opt/trn_rl_repo")

B, C, H, W = 32, 64, 64, 64
P, D = 256, 64
NCORES = 8
IMGS = B // NCORES            # 4 images per core
HWTOK = H * W                 # 4096 tokens per image
NTILE = HWTOK // 128          # 32
NSPAN = HWTOK // 512          # 8
NLVL = 8

# T1 row layout: list of (level, code) per row; pad rows get level -1
# rows 0-31: l4; rows 32-95: l5; rows 96-125: l0..l3; 126-127 pad
_T1_ROWS = [(4, j) for j in range(32)]
_T1_ROWS += [(5, j) for j in range(64)]
for lvl, k in ((0, 2), (1, 4), (2, 8), (3, 16)):
    _T1_ROWS += [(lvl, j) for j in range(k)]
_T1_ROWS += [(-1, 0), (-1, 0)]                     # rows 126-127 pad
assert len(_T1_ROWS) == 128


def _tile_maps():
    t1l = np.array([r[0] for r in _T1_ROWS])
    t1c = np.array([r[1] for r in _T1_ROWS])
    return [
        (t1l, t1c),
        (np.full(128, 6), np.arange(128)),            # T2: level 6
        (np.full(128, 7), np.arange(128)),            # T3: level 7 lo
        (np.full(128, 7), np.arange(128) + 128),      # T4: level 7 hi
    ]


def _host_consts(codebook):
    cb = np.ascontiguousarray(codebook, dtype=np.float32)         # [256, 64]
    esq = np.sum(cb * cb, axis=1, dtype=np.float32)               # [256]
    cbt2e = np.concatenate([-2.0 * cb.T, esq[None, :]], axis=0)   # [65, 256]
    cbt2e = np.ascontiguousarray(cbt2e, dtype=np.float32)

    iota_neg = np.broadcast_to(
        (np.arange(256, dtype=np.float32) - 512.0)[None, :], (128, 256)
    ).copy()

    selqs, biases, lts = [], [], []
    for rowlevel, rowcode in _tile_maps():
        # selq rows: slots 4l+0..2 = (idx_l, ihi_l, ilo_l) coefficients
        # where idx^2 = 256*ihi + ilo keeps every f32r operand <= 2^13.
        sq = np.zeros((32, 128), np.float32)
        for p_ in range(128):
            l = rowlevel[p_]
            if l >= 0:
                sq[4 * l + 0, p_] = -2.0 * rowcode[p_]   # coeff of idx_l
                sq[4 * l + 1, p_] = 256.0                # coeff of ihi_l
                sq[4 * l + 2, p_] = 1.0                  # coeff of ilo_l
        selqs.append(sq)
        code = rowcode.astype(np.float32).copy()
        code[rowlevel < 0] = 2.0   # pad sentinel: g2=0 -> 1-4<0 -> oh=0
        biases.append((1.0 - code * code).astype(np.float32))     # ACT Relu
        lts.append((0.5 - code * code).astype(np.float32))        # DVE is_lt

    # gather lhsT weights (zero rows where the onehot row isn't contracted)
    g01 = np.zeros((128, 128), np.float32)   # rows 96-127 used (base match)
    g01[96:98, 0:64] = cb[0:2]        # l0 -> out cols 0:64
    g01[98:102, 64:128] = cb[0:4]     # l1 -> out cols 64:128
    g23 = np.zeros((128, 128), np.float32)
    g23[102:110, 0:64] = cb[0:8]      # l2
    g23[110:126, 64:128] = cb[0:16]   # l3
    g45 = np.zeros((96, 128), np.float32)
    g45[0:32, 0:64] = cb[0:32]        # l4 (T1 rows 0-31)
    g45[32:96, 64:128] = cb[0:64]     # l5 (T1 rows 32-95)
    c67a = np.zeros((128, 128), np.float32)
    c67a[:, 0:64] = cb[0:128]                 # l6 -> out cols 0:64
    c67b = np.zeros((128, 128), np.float32)
    c67b[:, 64:128] = cb[0:128]               # l7 lo -> out cols 64:128
    c67c = np.zeros((128, 128), np.float32)
    c67c[:, 64:128] = cb[128:256]             # l7 hi -> out cols 64:128
    ident = np.eye(128, dtype=np.float32)
    onesrow = np.ones((1, HWTOK), np.float32)
    return dict(
        onesrow=onesrow,
        cbt2e=cbt2e, iota_neg=iota_neg,
        selq1=selqs[0], selq2=selqs[1], selq3=selqs[2], selq4=selqs[3],
        bias1=biases[0].reshape(128, 1), bias2=biases[1].reshape(128, 1),
        lt3=lts[2].reshape(128, 1), lt4=lts[3].reshape(128, 1),
        g01=g01, g23=g23, g45=g45, c67a=c67a, c67b=c67b, c67c=c67c,
        ident=ident,
    )


_CACHE = {}

# block b covers codes [blo, bhi)
_BLOCKS = [(0, 2)] + [(1 << b, 1 << (b + 1)) for b in range(1, 8)]


def _build():
    if "nc" in _CACHE:
        return _CACHE["nc"]
    from concourse import bass, bacc, tile, mybir

    f32 = mybir.dt.float32
    f32r = mybir.dt.float32r
    R_CONSTS = {"selq1", "selq2", "selq3", "selq4",
                "g01", "g23", "g45", "c67a", "c67b", "c67c"}
    def r(ap):
        return ap if ap.dtype == f32r else ap.bitcast(f32r)
    Alu = mybir.AluOpType
    Act = mybir.ActivationFunctionType
    AxX = mybir.AxisListType.X

    nc = bacc.Bacc("TRN2", target_bir_lowering=False, debug=False,
                   num_devices=NCORES)
    x_d = nc.declare_dram_parameter("x", [IMGS, 64, HWTOK], f32,
                                    isOutput=False)
    consts_spec = [
        ("cbt2e", [65, 256]), ("iota_neg", [128, 256]),
        ("selq1", [32, 128]), ("selq2", [32, 128]),
        ("selq3", [32, 128]), ("selq4", [32, 128]),
        ("bias1", [128, 1]), ("bias2", [128, 1]),
        ("lt3", [128, 1]), ("lt4", [128, 1]),
        ("g01", [128, 128]), ("g23", [128, 128]), ("g45", [96, 128]),
        ("c67a", [128, 128]), ("c67b", [128, 128]), ("c67c", [128, 128]),
        ("ident", [128, 128]), ("onesrow", [1, HWTOK]),
    ]
    cd = {name: nc.declare_dram_parameter(
              name, shp, f32r if name in R_CONSTS else f32, isOutput=False)
          for name, shp in consts_spec}
    out_d = nc.declare_dram_parameter("out", [NLVL, IMGS, 64, HWTOK], f32,
                                      isOutput=True)

    with tile.TileContext(nc) as tc:
        with (
            tc.tile_pool(name="const", bufs=1) as cpool,
            tc.tile_pool(name="xp", bufs=2) as xpool,
            tc.tile_pool(name="kb", bufs=2) as kbpool,
            tc.tile_pool(name="scr", bufs=1) as scrpool,
            tc.tile_pool(name="small", bufs=2) as smpool,
            tc.tile_pool(name="ohp", bufs=2) as ohpool,
            tc.tile_pool(name="outp", bufs=3) as outpool,
            tc.tile_pool(name="kps", bufs=2, space="PSUM") as kpsum,
            tc.tile_pool(name="tps", bufs=1, space="PSUM") as tpsum,
            tc.tile_pool(name="gqs", bufs=2, space="PSUM") as gqpsum,
            tc.tile_pool(name="gos", bufs=3, space="PSUM") as gopsum,
        ):
            cs = {}
            for name, shp in consts_spec:
                t = cpool.tile(shp, f32r if name in R_CONSTS else f32,
                               tag=name)
                nc.sync.dma_start(out=t[:], in_=cd[name][:])
                cs[name] = t

            for img in range(IMGS):
                xT = xpool.tile([65, HWTOK], f32)
                nc.sync.dma_start(out=xT[0:64, :], in_=x_d[img])
                nc.sync.dma_start(out=xT[64:65, :], in_=cs["onesrow"][:])

                keybuf = kbpool.tile([128, NTILE, 256], f32)
                for tt in range(NTILE // 2):
                    kp = kpsum.tile([128, 512], f32)
                    for j in range(2):
                        t = 2 * tt + j
                        nc.tensor.matmul(
                            kp[:, j * 256:(j + 1) * 256],
                            xT[:, t * 128:(t + 1) * 128], cs["cbt2e"][:],
                            start=True, stop=True,
                        )
                    nc.scalar.copy(keybuf[:, 2 * tt:2 * tt + 2, :], kp[:])

                bm = smpool.tile([128, 8, NTILE], f32, tag="bm")
                bidx = smpool.tile([128, 8, NTILE], f32, tag="bidx")
                eqs = scrpool.tile([128, NTILE, 128], f32)
                for b, (blo, bhi) in enumerate(_BLOCKS):
                    h = bhi - blo
                    nc.vector.tensor_reduce(
                        out=bm[:, b, :], in_=keybuf[:, :, blo:bhi],
                        axis=AxX, op=Alu.min,
                    )
                    bmb = bm[:, b, :].unsqueeze(2).to_broadcast([128, NTILE, h])
                    nc.vector.tensor_tensor(
                        out=eqs[:, :, 0:h], in0=keybuf[:, :, blo:bhi],
                        in1=bmb, op=Alu.is_equal,
                    )
                    iot = cs["iota_neg"][:, blo:bhi].unsqueeze(1).to_broadcast(
                        [128, NTILE, h])
                    nc.gpsimd.tensor_tensor(
                        out=eqs[:, :, 0:h], in0=eqs[:, :, 0:h], in1=iot,
                        op=Alu.mult,
                    )
                    nc.vector.tensor_reduce(
                        out=bidx[:, b, :], in_=eqs[:, :, 0:h],
                        axis=AxX, op=Alu.min,
                    )

                # running chain over blocks -> per-level idx (and idx^2),
                # written into 32-slot-per-tile layout for the transpose
                # (slots 2l = idx_l, 2l+1 = idx_l^2, 16..31 pad).
                idxq = smpool.tile([128, NTILE, 32], f32, tag="idxq")
                nc.gpsimd.memset(idxq[:], 0.0)
                runmin = bm[:, 0, :]
                runidx = bidx[:, 0, :]
                nc.vector.tensor_scalar(
                    out=idxq[:, :, 0], in0=bidx[:, 0, :],
                    scalar1=512.0, scalar2=None, op0=Alu.add,
                )  # slot 4*0 == 0
                for l in range(1, NLVL):
                    mask = smpool.tile([128, NTILE], mybir.dt.int32,
                                       tag="mask")
                    nm = smpool.tile([128, NTILE], f32, tag="nm")
                    ni = smpool.tile([128, NTILE], f32, tag="ni")
                    nc.vector.tensor_tensor(out=mask[:], in0=bm[:, l, :],
                                            in1=runmin, op=Alu.is_lt)
                    nc.vector.tensor_tensor(out=nm[:], in0=bm[:, l, :],
                                            in1=runmin, op=Alu.min)
                    nc.vector.tensor_copy(ni[:], runidx)
                    nc.vector.copy_predicated(ni[:], mask[:], bidx[:, l, :])
                    nc.vector.tensor_scalar(
                        out=idxq[:, :, 4 * l], in0=ni[:],
                        scalar1=512.0, scalar2=None, op0=Alu.add,
                    )
                    runmin, runidx = nm[:], ni[:]
                # idx^2 = 256*rhi + rlo (rhi = round(sq/256) via 2^23 magic,
                # |rlo| <= 128) keeps every f32r operand <= 2^13 exact.
                # slots: 0 idx, 1 rhi, 2 rlo, 3 scratch (sq then u then rhi256)
                nc.vector.tensor_tensor(
                    out=idxq[:, :, 3:32:4], in0=idxq[:, :, 0:32:4],
                    in1=idxq[:, :, 0:32:4], op=Alu.mult,
                )
                MAGIC = float(2 ** 23)
                nc.vector.tensor_scalar(
                    out=idxq[:, :, 3:32:4], in0=idxq[:, :, 3:32:4],
                    scalar1=1.0 / 256.0, scalar2=MAGIC,
                    op0=Alu.mult, op1=Alu.add,
                )
                nc.vector.tensor_scalar(
                    out=idxq[:, :, 1:32:4], in0=idxq[:, :, 3:32:4],
                    scalar1=-MAGIC, scalar2=None, op0=Alu.add,
                )
                nc.vector.tensor_scalar(
                    out=idxq[:, :, 3:32:4], in0=idxq[:, :, 1:32:4],
                    scalar1=256.0, scalar2=None, op0=Alu.mult,
                )
                nc.vector.tensor_tensor(
                    out=idxq[:, :, 2:32:4], in0=idxq[:, :, 0:32:4],
                    in1=idxq[:, :, 0:32:4], op=Alu.mult,
                )
                nc.vector.tensor_tensor(
                    out=idxq[:, :, 2:32:4], in0=idxq[:, :, 2:32:4],
                    in1=idxq[:, :, 3:32:4], op=Alu.subtract,
                )

                for s in range(NSPAN):
                    tp = tpsum.tile([128, 128], f32)
                    nc.tensor.transpose(
                        tp[:], idxq[:, 4 * s:4 * s + 4, :], cs["ident"][:],
                    )
                    R = smpool.tile([32, 512], f32r, tag="R")
                    for t in range(4):
                        nc.vector.tensor_copy(
                            R[0:32, t * 128:(t + 1) * 128],
                            tp[32 * t:32 * (t + 1), :],
                        )
                    ohs = {}
                    for X, selq in ((1, "selq1"), (2, "selq2"),
                                    (3, "selq3"), (4, "selq4")):
                        gq = gqpsum.tile([128, 512], f32)
                        nc.tensor.matmul(gq[:], r(cs[selq][:]), r(R[:]),
                                         start=True, stop=True)
                        oht = ohpool.tile([128, 512], f32r, tag=f"oh{X}")
                        if X <= 2:
                            nc.scalar.activation(oht[:], gq[:], Act.Relu,
                                                 bias=cs[f"bias{X}"][:],
                                                 scale=-1.0)
                        else:
                            nc.vector.tensor_scalar(
                                out=oht[:], in0=gq[:], scalar1=cs[f"lt{X}"][:],
                                scalar2=None, op0=Alu.is_lt,
                            )
                        ohs[X] = oht

                    ps01 = gopsum.tile([128, 512], f32, tag="go")
                    ps23 = gopsum.tile([128, 512], f32, tag="go")
                    ps45 = gopsum.tile([128, 512], f32, tag="go")
                    ps67 = gopsum.tile([128, 512], f32, tag="go")
                    nc.tensor.matmul(ps01[:], r(cs["g01"][96:128, :]),
                                     r(ohs[1][96:128, :]),
                                     start=True, stop=True,
                                     tile_position=(96, 0))
                    nc.tensor.matmul(ps23[:], r(cs["g23"][96:128, :]),
                                     r(ohs[1][96:128, :]),
                                     start=True, stop=True,
                                     tile_position=(96, 0))
                    nc.tensor.matmul(ps45[:], r(cs["g45"][:]),
                                     r(ohs[1][0:96, :]),
                                     start=True, stop=True)
                    nc.tensor.matmul(ps67[:], r(cs["c67a"][:]),
                                     r(ohs[2][:]), start=True, stop=False)
                    nc.tensor.matmul(ps67[:], r(cs["c67b"][:]),
                                     r(ohs[3][:]), start=False, stop=False)
                    nc.tensor.matmul(ps67[:], r(cs["c67c"][:]),
                                     r(ohs[4][:]), start=False, stop=True)

                    for pi, (ps, eng) in enumerate(
                        ((ps01, "act"), (ps23, "act"),
                         (ps45, "dve"), (ps67, "dve"))
                    ):
                        cp = outpool.tile([128, 512], f32, tag=f"cp{pi}")
                        if eng == "act":
                            nc.scalar.copy(cp[:], ps[:])
                        else:
                            nc.vector.tensor_copy(cp[:], ps[:])
                        for half in range(2):
                            lvl = 2 * pi + half
                            nc.sync.dma_start(
                                out=out_d[lvl, img, :,
                                          s * 512:(s + 1) * 512],
                                in_=cp[64 * half:64 * (half + 1), :],
                            )
    nc.compile()
    _CACHE["nc"] = nc
    return nc


def kernel(input_data, codebook, previous_active_vectors=None,
           num_active_vectors=256, **_):
    from concourse.bass_utils import run_bass_kernel_spmd

    x = np.ascontiguousarray(np.asarray(input_data, dtype=np.float32))
    assert x.shape == (B, C, H, W)
    consts = _host_consts(np.asarray(codebook, dtype=np.float32))

    nc = _build()
    in_maps = []
    for core in range(NCORES):
        m = {"x": np.ascontiguousarray(
            x[core * IMGS:(core + 1) * IMGS].reshape(IMGS, 64, HWTOK))}
        m.update(consts)
        in_maps.append(m)
    res = run_bass_kernel_spmd(nc, in_maps, core_ids=list(range(NCORES)))
    outs = [res.results[i]["out"] for i in range(NCORES)]   # [8, 4, 64, 4096]
    full = np.concatenate(outs, axis=1)                     # [8, 32, 64, 4096]
    return full.reshape(NLVL, B, C, H, W)


# revision 20
# speedup vs baseline: 1.8237x; 1.0291x over previous
"""AdaptiveVectorQuantizer Trainium2 kernel (8 NeuronCores, data-parallel).

Layout insight: BCHW input means each image b is natively [C=64, H*W=4096] =
channels-on-partitions, tokens-on-free — exactly the xT layout the PE wants,
and the output [L, B, C, H, W] is also channel-major, so gather results
[64, T] DMA out with 2KB-contiguous runs. No transposes of bulk data.

Per core: 4 images. Per image (4096 tokens, 32 tiles of 128):
 1. PE: score psum [128 tok, 256 codes] = e_sq - 2 x.e  (K=65 matmul: x rows
    + a ones row; rhs = [-2*cbT ; e_sq]).  argmin(score) == argmin dist.
 2. ACT copies psum -> packed SBUF keybuf [128, 32, 256].
 3. DVE/GP: per dyadic block b (sizes 2,2,4,..,128 covering codes 0..255):
    block-min (free-axis reduce), eq-mask, mask*iota_neg, reduce -> block
    argmin (first-occurrence ties).  Running chain over blocks yields
    per-level argmin idx_l (levels l: prefix k=2^(l+1)).
 4. PE transpose bridges idx (+idx^2) to rows; K=32 selector matmul computes
    g2 = idx^2 - 2*code*idx per combined-onehot row; threshold
    Relu(1-code^2 - g2) / is_lt(0.5-code^2) gives exact {0,1} one-hots.
 5. Block-diagonal gather matmuls (zero rows pad to 32-aligned slices)
    produce channel-major outputs for level pairs; psum->sbuf; DMA out.

Combined-onehot tiles (rows must be 32-aligned for engine slicing):
  T1: rows 0-1 l0 | 2-5 l1 | 6-13 l2 | 14-29 l3 | 30-31 pad | 32-63 l4
      | 64-127 l5
  T2: l6 codes 0-127.  T3: l7 codes 0-127.  T4: l7 codes 128-255.
"""

import sys, os
import numpy as np

sys.path.insert(0, "/opt/trn_rl_repo")

B, C, H, W = 32, 64, 64, 64
P, D = 256, 64
NCORES = 8
IMGS = B // NCORES            # 4 images per core
HWTOK = H * W                 # 4096 tokens per image
NTILE = HWTOK // 128          # 32
NSPAN = HWTOK // 512          # 8
NLVL = 8

# T1 row layout: list of (level, code) per row; pad rows get level -1
# rows 0-31: l4; rows 32-95: l5; rows 96-125: l0..l3; 126-127 pad
_T1_ROWS = [(4, j) for j in range(32)]
_T1_ROWS += [(5, j) for j in range(64)]
for lvl, k in ((0, 2), (1, 4), (2, 8), (3, 16)):
    _T1_ROWS += [(lvl, j) for j in range(k)]
_T1_ROWS += [(-1, 0), (-1, 0)]                     # rows 126-127 pad
assert len(_T1_ROWS) == 128


def _tile_maps():
    t1l = np.array([r[0] for r in _T1_ROWS])
    t1c = np.array([r[1] for r in _T1_ROWS])
    return [
        (t1l, t1c),
        (np.full(128, 6), np.arange(128)),            # T2: level 6
        (np.full(128, 7), np.arange(128)),            # T3: level 7 lo
        (np.full(128, 7), np.arange(128) + 128),      # T4: level 7 hi
    ]


def _host_consts(codebook):
    cb = np.ascontiguousarray(codebook, dtype=np.float32)         # [256, 64]
    esq = np.sum(cb * cb, axis=1, dtype=np.float32)               # [256]
    cbt2e = np.concatenate([-2.0 * cb.T, esq[None, :]], axis=0)   # [65, 256]
    cbt2e = np.ascontiguousarray(cbt2e, dtype=np.float32)

    iota_neg = np.broadcast_to(
        (np.arange(256, dtype=np.float32) - 512.0)[None, :], (128, 256)
    ).copy()

    selqs, biases, lts = [], [], []
    for rowlevel, rowcode in _tile_maps():
        # selq rows (q-major slots): r = q*8 + l with q = 0 idx, 1 rhi,
        # 2 rlo, 3 scratch; idx^2 = 256*rhi + rlo keeps f32r operands exact.
        sq = np.zeros((32, 128), np.float32)
        for p_ in range(128):
            l = rowlevel[p_]
            if l >= 0:
                sq[l, p_] = -2.0 * rowcode[p_]      # coeff of idx_l
                sq[8 + l, p_] = 256.0               # coeff of rhi_l
                sq[16 + l, p_] = 1.0                # coeff of rlo_l
        selqs.append(sq)
        code = rowcode.astype(np.float32).copy()
        code[rowlevel < 0] = 2.0   # pad sentinel: g2=0 -> 1-4<0 -> oh=0
        biases.append((1.0 - code * code).astype(np.float32))     # ACT Relu
        lts.append((0.5 - code * code).astype(np.float32))        # DVE is_lt

    # gather lhsT weights (zero rows where the onehot row isn't contracted)
    g01 = np.zeros((128, 128), np.float32)   # rows 96-127 used (base match)
    g01[96:98, 0:64] = cb[0:2]        # l0 -> out cols 0:64
    g01[98:102, 64:128] = cb[0:4]     # l1 -> out cols 64:128
    g23 = np.zeros((128, 128), np.float32)
    g23[102:110, 0:64] = cb[0:8]      # l2
    g23[110:126, 64:128] = cb[0:16]   # l3
    g45 = np.zeros((96, 128), np.float32)
    g45[0:32, 0:64] = cb[0:32]        # l4 (T1 rows 0-31)
    g45[32:96, 64:128] = cb[0:64]     # l5 (T1 rows 32-95)
    c67a = np.zeros((128, 128), np.float32)
    c67a[:, 0:64] = cb[0:128]                 # l6 -> out cols 0:64
    c67b = np.zeros((128, 128), np.float32)
    c67b[:, 64:128] = cb[0:128]               # l7 lo -> out cols 64:128
    c67c = np.zeros((128, 128), np.float32)
    c67c[:, 64:128] = cb[128:256]             # l7 hi -> out cols 64:128
    ident = np.eye(128, dtype=np.float32)
    onesrow = np.ones((1, HWTOK), np.float32)
    return dict(
        onesrow=onesrow,
        cbt2e=cbt2e, iota_neg=iota_neg,
        selq1=selqs[0], selq2=selqs[1], selq3=selqs[2], selq4=selqs[3],
        bias1=biases[0].reshape(128, 1), bias2=biases[1].reshape(128, 1),
        bias3=biases[2].reshape(128, 1), lt4=lts[3].reshape(128, 1),
        g01=g01, g23=g23, g45=g45, c67a=c67a, c67b=c67b, c67c=c67c,
        ident=ident,
    )


_CACHE = {}

# block b covers codes [blo, bhi)
_BLOCKS = [(0, 2)] + [(1 << b, 1 << (b + 1)) for b in range(1, 8)]


def _build():
    if "nc" in _CACHE:
        return _CACHE["nc"]
    from concourse import bass, bacc, tile, mybir

    f32 = mybir.dt.float32
    f32r = mybir.dt.float32r
    R_CONSTS = {"selq1", "selq2", "selq3", "selq4",
                "g01", "g23", "g45", "c67a", "c67b", "c67c"}
    def r(ap):
        return ap if ap.dtype == f32r else ap.bitcast(f32r)
    Alu = mybir.AluOpType
    Act = mybir.ActivationFunctionType
    AxX = mybir.AxisListType.X

    nc = bacc.Bacc("TRN2", target_bir_lowering=False, debug=False,
                   num_devices=NCORES)
    x_d = nc.declare_dram_parameter("x", [IMGS, 64, HWTOK], f32,
                                    isOutput=False)
    consts_spec = [
        ("cbt2e", [65, 256]), ("iota_neg", [128, 256]),
        ("selq1", [32, 128]), ("selq2", [32, 128]),
        ("selq3", [32, 128]), ("selq4", [32, 128]),
        ("bias1", [128, 1]), ("bias2", [128, 1]),
        ("bias3", [128, 1]), ("lt4", [128, 1]),
        ("g01", [128, 128]), ("g23", [128, 128]), ("g45", [96, 128]),
        ("c67a", [128, 128]), ("c67b", [128, 128]), ("c67c", [128, 128]),
        ("ident", [128, 128]), ("onesrow", [1, HWTOK]),
    ]
    cd = {name: nc.declare_dram_parameter(
              name, shp, f32r if name in R_CONSTS else f32, isOutput=False)
          for name, shp in consts_spec}
    out_d = nc.declare_dram_parameter("out", [NLVL, IMGS, 64, HWTOK], f32,
                                      isOutput=True)

    with tile.TileContext(nc) as tc:
        with (
            tc.tile_pool(name="const", bufs=1) as cpool,
            tc.tile_pool(name="xp", bufs=2) as xpool,
            tc.tile_pool(name="kb", bufs=2) as kbpool,
            tc.tile_pool(name="scr", bufs=1) as scrpool,
            tc.tile_pool(name="small", bufs=2) as smpool,
            tc.tile_pool(name="ohp", bufs=2) as ohpool,
            tc.tile_pool(name="outp", bufs=3) as outpool,
            tc.tile_pool(name="kps", bufs=2, space="PSUM") as kpsum,
            tc.tile_pool(name="tps", bufs=1, space="PSUM") as tpsum,
            tc.tile_pool(name="gqs", bufs=2, space="PSUM") as gqpsum,
            tc.tile_pool(name="gos", bufs=3, space="PSUM") as gopsum,
        ):
            cs = {}
            for name, shp in consts_spec:
                t = cpool.tile(shp, f32r if name in R_CONSTS else f32,
                               tag=name)
                nc.sync.dma_start(out=t[:], in_=cd[name][:])
                cs[name] = t

            for img in range(IMGS):
                xT = xpool.tile([65, HWTOK], f32)
                nc.sync.dma_start(out=xT[0:64, :], in_=x_d[img])
                nc.sync.dma_start(out=xT[64:65, :], in_=cs["onesrow"][:])

                keybuf = kbpool.tile([128, NTILE, 256], f32)
                for tt in range(NTILE // 2):
                    kp = kpsum.tile([128, 512], f32)
                    for j in range(2):
                        t = 2 * tt + j
                        nc.tensor.matmul(
                            kp[:, j * 256:(j + 1) * 256],
                            xT[:, t * 128:(t + 1) * 128], cs["cbt2e"][:],
                            start=True, stop=True,
                        )
                    if tt % 2 == 0:
                        nc.scalar.copy(keybuf[:, 2 * tt:2 * tt + 2, :], kp[:])
                    else:
                        nc.vector.tensor_copy(
                            keybuf[:, 2 * tt:2 * tt + 2, :], kp[:])

                bm = smpool.tile([128, 8, NTILE], f32, tag="bm")
                bidx = smpool.tile([128, 8, NTILE], f32, tag="bidx")
                eqs = scrpool.tile([128, NTILE, 128], f32)
                for b, (blo, bhi) in enumerate(_BLOCKS):
                    h = bhi - blo
                    nc.vector.tensor_reduce(
                        out=bm[:, b, :], in_=keybuf[:, :, blo:bhi],
                        axis=AxX, op=Alu.min,
                    )
                    bmb = bm[:, b, :].unsqueeze(2).to_broadcast([128, NTILE, h])
                    nc.vector.tensor_tensor(
                        out=eqs[:, :, 0:h], in0=keybuf[:, :, blo:bhi],
                        in1=bmb, op=Alu.is_equal,
                    )
                    iot = cs["iota_neg"][:, blo:bhi].unsqueeze(1).to_broadcast(
                        [128, NTILE, h])
                    nc.gpsimd.tensor_tensor(
                        out=eqs[:, :, 0:h], in0=eqs[:, :, 0:h], in1=iot,
                        op=Alu.mult,
                    )
                    nc.vector.tensor_reduce(
                        out=bidx[:, b, :], in_=eqs[:, :, 0:h],
                        axis=AxX, op=Alu.min,
                    )

                # running chain over blocks -> per-level idx; slot-major
                # layout [128, slot, tile]: slots l=idx, 8+l=rhi, 16+l=rlo,
                # 24+l=scratch -- all chain/pipeline writes contiguous.
                scr8 = smpool.tile([128, 32, NTILE], f32, tag="scr8")
                idxq = smpool.tile([128, NTILE, 32], f32, tag="idxq")
                runmin = bm[:, 0, :]
                runidx = bidx[:, 0, :]
                nc.vector.tensor_scalar(
                    out=scr8[:, 0, :], in0=bidx[:, 0, :],
                    scalar1=512.0, scalar2=None, op0=Alu.add,
                )
                for l in range(1, NLVL):
                    mask = smpool.tile([128, NTILE], mybir.dt.int32,
                                       tag="mask")
                    nm = smpool.tile([128, NTILE], f32, tag="nm")
                    ni = smpool.tile([128, NTILE], f32, tag="ni")
                    nc.vector.tensor_tensor(out=mask[:], in0=bm[:, l, :],
                                            in1=runmin, op=Alu.is_lt)
                    nc.vector.tensor_tensor(out=nm[:], in0=bm[:, l, :],
                                            in1=runmin, op=Alu.min)
                    nc.vector.tensor_copy(ni[:], runidx)
                    nc.vector.copy_predicated(ni[:], mask[:], bidx[:, l, :])
                    nc.vector.tensor_scalar(
                        out=scr8[:, l, :], in0=ni[:],
                        scalar1=512.0, scalar2=None, op0=Alu.add,
                    )
                    runmin, runidx = nm[:], ni[:]
                # idx^2 = 256*rhi + rlo (rhi = round(sq/256) via 2^23
                # magic, |rlo| <= 128): all slabs contiguous [128, 8, NTILE].
                MAGIC = float(2 ** 23)
                nc.vector.tensor_tensor(
                    out=scr8[:, 24:32, :], in0=scr8[:, 0:8, :],
                    in1=scr8[:, 0:8, :], op=Alu.mult,
                )  # sq -> scratch slab
                nc.vector.tensor_scalar(
                    out=scr8[:, 8:16, :], in0=scr8[:, 24:32, :],
                    scalar1=1.0 / 256.0, scalar2=MAGIC,
                    op0=Alu.mult, op1=Alu.add,
                )  # u
                nc.vector.tensor_scalar(
                    out=scr8[:, 8:16, :], in0=scr8[:, 8:16, :],
                    scalar1=-MAGIC, scalar2=None, op0=Alu.add,
                )  # rhi
                nc.vector.tensor_scalar(
                    out=scr8[:, 16:24, :], in0=scr8[:, 8:16, :],
                    scalar1=256.0, scalar2=None, op0=Alu.mult,
                )  # rhi*256
                nc.vector.tensor_tensor(
                    out=scr8[:, 16:24, :], in0=scr8[:, 24:32, :],
                    in1=scr8[:, 16:24, :], op=Alu.subtract,
                )  # rlo = sq - 256*rhi
                nc.vector.tensor_copy(
                    idxq[:], scr8[:].transpose([0, 2, 1]))

                for s in range(NSPAN):
                    tp = tpsum.tile([128, 128], f32)
                    nc.tensor.transpose(
                        tp[:], idxq[:, 4 * s:4 * s + 4, :], cs["ident"][:],
                    )
                    R = smpool.tile([32, 512], f32r, tag="R")
                    for t in range(4):
                        nc.vector.tensor_copy(
                            R[0:32, t * 128:(t + 1) * 128],
                            tp[32 * t:32 * (t + 1), :],
                        )
                    ohs = {}
                    for X, selq in ((1, "selq1"), (2, "selq2"),
                                    (3, "selq3"), (4, "selq4")):
                        gq = gqpsum.tile([128, 512], f32)
                        nc.tensor.matmul(gq[:], r(cs[selq][:]), r(R[:]),
                                         start=True, stop=True)
                        oht = ohpool.tile([128, 512], f32r, tag=f"oh{X}")
                        if X <= 3:
                            nc.scalar.activation(oht[:], gq[:], Act.Relu,
                                                 bias=cs[f"bias{X}"][:],
                                                 scale=-1.0)
                        else:
                            nc.vector.tensor_scalar(
                                out=oht[:], in0=gq[:], scalar1=cs[f"lt{X}"][:],
                                scalar2=None, op0=Alu.is_lt,
                            )
                        ohs[X] = oht

                    ps01 = gopsum.tile([128, 512], f32, tag="go")
                    ps23 = gopsum.tile([128, 512], f32, tag="go")
                    ps45 = gopsum.tile([128, 512], f32, tag="go")
                    ps67 = gopsum.tile([128, 512], f32, tag="go")
                    nc.tensor.matmul(ps01[:], r(cs["g01"][96:128, :]),
                                     r(ohs[1][96:128, :]),
                                     start=True, stop=True,
                                     tile_position=(96, 0))
                    nc.tensor.matmul(ps23[:], r(cs["g23"][96:128, :]),
                                     r(ohs[1][96:128, :]),
                                     start=True, stop=True,
                                     tile_position=(96, 0))
                    nc.tensor.matmul(ps45[:], r(cs["g45"][:]),
                                     r(ohs[1][0:96, :]),
                                     start=True, stop=True)
                    nc.tensor.matmul(ps67[:], r(cs["c67a"][:]),
                                     r(ohs[2][:]), start=True, stop=False)
                    nc.tensor.matmul(ps67[:], r(cs["c67b"][:]),
                                     r(ohs[3][:]), start=False, stop=False)
                    nc.tensor.matmul(ps67[:], r(cs["c67c"][:]),
                                     r(ohs[4][:]), start=False, stop=True)

                    for pi, (ps, eng) in enumerate(
                        ((ps01, "act"), (ps23, "act"),
                         (ps45, "dve"), (ps67, "dve"))
                    ):
                        cp = outpool.tile([128, 512], f32, tag=f"cp{pi}")
                        if eng == "act":
                            nc.scalar.copy(cp[:], ps[:])
                        else:
                            nc.vector.tensor_copy(cp[:], ps[:])
                        for half in range(2):
                            lvl = 2 * pi + half
                            nc.sync.dma_start(
                                out=out_d[lvl, img, :,
                                          s * 512:(s + 1) * 512],
                                in_=cp[64 * half:64 * (half + 1), :],
                            )
    nc.compile()
    _CACHE["nc"] = nc
    return nc


def kernel(input_data, codebook, previous_active_vectors=None,
           num_active_vectors=256, **_):
    from concourse.bass_utils import run_bass_kernel_spmd

    x = np.ascontiguousarray(np.asarray(input_data, dtype=np.float32))
    assert x.shape == (B, C, H, W)
    consts = _host_consts(np.asarray(codebook, dtype=np.float32))

    nc = _build()
    in_maps = []
    for core in range(NCORES):
        m = {"x": np.ascontiguousarray(
            x[core * IMGS:(core + 1) * IMGS].reshape(IMGS, 64, HWTOK))}
        m.update(consts)
        in_maps.append(m)
    res = run_bass_kernel_spmd(nc, in_maps, core_ids=list(range(NCORES)))
    outs = [res.results[i]["out"] for i in range(NCORES)]   # [8, 4, 64, 4096]
    full = np.concatenate(outs, axis=1)                     # [8, 32, 64, 4096]
    return full.reshape(NLVL, B, C, H, W)
